# revision 14
# baseline (speedup 1.0000x reference)
"""StyleGAN2 fused upsample2x + 3x3 conv + FIR(1,3,3,1) + bias — TRN2 Bass kernel.

Decomposition: per dimension the reference is zero-insert by 2 -> correlate w
(full pad) -> pad 1 -> correlate FIR f1=(1,3,3,1)/4 (gain 2/dim).  f1 factors
as box^3/4 with box=(1,1).  We compose ONE box into w horizontally
(wh = w *h (1,1), width 4) and leave box^2=(1,2,1) horizontally plus box^3
vertically as cheap DVE adjacent-add cascades in fp16 (2x mode).

The conv produces the dense fine grid g[gt, gs] (132x130 alloc, 131x130 used):
  g rows: gt = 2t   (even)  = o-plane: w row 1   on x row u,  t = u+1
          gt = 2t+1 (odd)   = e-plane: w rows 0,2 on x rows u-1,u, t = u
  g cols: gs = 2v'+bb (g col gs = G[gs-1], G[t] = sum_j wh[j] xz[t+j-2]):
          bb=0 taps wh{1,3}, bb=1 taps wh{0,2}; both on x cols v'+jj-1
Then out[r,s] = sum (1,2,1)_h (1,3,3,1)_v g[r..r+3, s..s+2] (cascaded adds).
Weight scale 1/16 and bias/32 are folded in (h box^2 sums 4, v box^3 sums 8).

Matmuls fp16 (1 cycle/row).  K=256 in 2x128, o=256 in 2x128.  PSUM chains of
7 rows x 65 cols (455 fp32 <= 512/bank).  Data-parallel: 2 images/core.
"""

import sys

sys.path.insert(0, "/opt/trn_rl_repo")

import numpy as np

import concourse.bacc as bacc
import concourse.mybir as mybir
import concourse.tile as tile
from concourse.bass_utils import run_bass_kernel_spmd

N_CORES = 8
IMGS = 16
IMG_PER_CORE = IMGS // N_CORES  # 2
C = 256
O = 256
H = W = 64
NK = C // 128  # 2
NM = O // 128  # 2
HPR = 66  # padded x rows (pad 1 top/bottom)
HPC = 66  # padded x cols (pad 1 left/right)
GC = 130  # g cols used (alloc 130)
NTAP = 12  # taps per (m, bb): 8 e-plane + 4 o-plane
NU = 7  # chain rows
NV = 65  # chain cols (one col parity)

_compiled = None
LAST_RESULTS = None


def _chains(img, m):
    """Chain specs in execution order for one (img, m) plane.

    chain = (bb, two, t0, nu, taps); taps = (wt_idx, k, row_lo, col_lo):
      rhs = xp[img, k][:, row_lo : row_lo + nu, col_lo : col_lo + 65]
      g rows: 2*t + two for t in t0 .. t0+nu-1; g cols bb::2
    """
    for j in range(10):
        for plane in ("o", "e"):
            for bb in range(2):
                base = (m * 2 + bb) * NTAP
                if plane == "o":
                    u0 = -1 + NU * j
                    nu = min(NU, 65 - u0)  # u in -1..64 (66 rows)
                    if nu <= 0:
                        continue
                    taps = [
                        (base + 8 + k * 2 + jj, k, u0 + 1, jj)
                        for k in range(NK)
                        for jj in range(2)
                    ]
                    yield (bb, 0, u0 + 1, nu, taps)
                else:
                    u0 = NU * j
                    nu = min(NU, 65 - u0)  # u in 0..64 (65 rows)
                    if nu <= 0:
                        continue
                    taps = [
                        (base + k * 4 + ds * 2 + jj, k, u0 + ds, jj)
                        for k in range(NK)
                        for ds in range(2)  # ds=0 -> w row 0 (x[u-1]); 1 -> w row 2
                        for jj in range(2)
                    ]
                    yield (bb, 1, u0, nu, taps)


def _build():
    nc = bacc.Bacc(None, target_bir_lowering=False, debug=False)
    dt = mybir.dt

    xp_d = nc.dram_tensor(
        "xp", (IMG_PER_CORE, NK, 128, HPR * HPC), dt.float16, kind="ExternalInput"
    )
    wt_d = nc.dram_tensor(
        "wt", (128, 4 * NTAP * 128), dt.float16, kind="ExternalInput"
    )
    b_d = nc.dram_tensor("bias", (128, NM), dt.float32, kind="ExternalInput")
    out_d = nc.dram_tensor(
        "out", (IMG_PER_CORE, O, 2 * H, 2 * W), dt.float16, kind="ExternalOutput"
    )

    with tile.TileContext(nc) as tc:
        with (
            tc.tile_pool(name="xpool", bufs=1) as xpool,
            tc.tile_pool(name="wpool", bufs=1) as wpool,
            tc.tile_pool(name="gpool", bufs=2) as gpool,
            tc.tile_pool(name="cpool", bufs=1) as cpool,
            tc.tile_pool(name="opool", bufs=3) as opool,
            tc.tile_pool(name="wpsum", bufs=1, space="PSUM") as wpsum_pool,
            tc.tile_pool(name="psum", bufs=7, space="PSUM") as psum_pool,
        ):
            wt_t = wpool.tile([128, 4 * NTAP * 128], dt.float16, tag="wt")
            b_t = wpool.tile([128, NM], dt.float32, tag="bias")
            xp_t = {}

            def load_xp(img, k, strips):
                if (img, k) not in xp_t:
                    xp_t[img, k] = xpool.tile(
                        [128, HPR, HPC], dt.float16, tag=f"xp{img}{k}",
                        name=f"xp{img}{k}",
                    )
                t = xp_t[img, k]
                src = xp_d.ap()[img, k].rearrange("p (h w) -> p h w", h=HPR)
                for lo, hi in strips:
                    nc.sync.dma_start(t[:, lo:hi, :], src[:, lo:hi, :])

            def load_wt(m):
                blk = m * 2 * NTAP * 128
                n = 2 * NTAP * 128
                nc.sync.dma_start(wt_t[:, blk : blk + n], wt_d.ap()[:, blk : blk + n])

            # PE warmup during DMA preamble: garbage matmuls ramp the
            # tensor engine p-state to full clock before real work arrives.
            warm_t = wpool.tile([128, 512], dt.float16, name="warm")
            nc.gpsimd.memset(warm_t[:], 0.0)
            warm_ps = wpsum_pool.tile([128, 512], dt.float32, name="warm_ps")
            for _ in range(14):
                nc.tensor.matmul(
                    warm_ps[:], warm_t[:, :128], warm_t[:], start=True, stop=True
                )
            # minimal working set first: j=0..1 chains need xp rows < 16
            load_wt(0)
            load_xp(0, 0, [(0, 16)])
            load_xp(0, 1, [(0, 16)])
            nc.sync.dma_start(b_t[:], b_d.ap()[:])
            load_xp(0, 0, [(16, HPR)])
            load_xp(0, 1, [(16, HPR)])
            load_wt(1)
            load_xp(1, 0, [(0, HPR)])
            load_xp(1, 1, [(0, HPR)])

            for img in range(IMG_PER_CORE):
                for m in range(NM):
                    g_t = gpool.tile([128, 132, GC], dt.float16, name="g")
                    g_v = g_t[:].rearrange(
                        "p (t two) (v bb) -> p two t v bb", two=2, bb=2
                    )
                    for bb, two, t0, nu, taps in _chains(img, m):
                        acc = psum_pool.tile([128, nu, NV], dt.float32, name="acc")
                        for i, (wt_idx, k, row_lo, col_lo) in enumerate(taps):
                            nc.tensor.matmul(
                                acc[:],
                                wt_t[:, wt_idx * 128 : (wt_idx + 1) * 128],
                                xp_t[img, k][:, row_lo : row_lo + nu, col_lo : col_lo + NV],
                                start=(i == 0),
                                stop=(i == len(taps) - 1),
                            )
                        nc.scalar.activation(
                            g_v[:, two, t0 : t0 + nu, :, bb],
                            acc[:],
                            mybir.ActivationFunctionType.Identity,
                            bias=b_t[:, m : m + 1],
                        )
                    # FIR cascades: v (1,3,3,1) then h (1,2,1), fp16 adds.
                    # bands = (r0, nrows); v2 of bands 0,2 runs on GpSimd.
                    last_unit = img == IMG_PER_CORE - 1 and m == NM - 1
                    bands = [(0, 32), (32, 32), (64, 32)] + (
                        [(96, 16), (112, 16)] if last_unit else [(96, 32)]
                    )
                    add = mybir.AluOpType.add
                    for bi, (r0, nr) in enumerate(bands):
                        p1 = cpool.tile([128, nr + 2, GC], dt.float16, name="casc_p1")
                        p2 = cpool.tile([128, nr + 1, GC], dt.float16, name="casc_p2")
                        p3 = cpool.tile([128, nr, GC], dt.float16, name="casc_p3")
                        p4 = cpool.tile([128, nr, 129], dt.float16, name="casc_p4")
                        o_t = opool.tile([128, nr, 128], dt.float16, name="casc_o")
                        nc.vector.tensor_tensor(
                            p1[:], g_t[:, r0 : r0 + nr + 2, :GC],
                            g_t[:, r0 + 1 : r0 + nr + 3, :GC], add,
                        )
                        eng = nc.gpsimd if bi in (0, 2) else nc.vector
                        eng.tensor_tensor(
                            p2[:], p1[:, 0 : nr + 1, :], p1[:, 1 : nr + 2, :], add,
                        )
                        nc.vector.tensor_tensor(
                            p3[:], p2[:, 0:nr, :], p2[:, 1 : nr + 1, :], add,
                        )
                        nc.vector.tensor_tensor(
                            p4[:], p3[:, :, 0:129], p3[:, :, 1:130], add,
                        )
                        nc.vector.tensor_tensor(
                            o_t[:], p4[:, :, 0:128], p4[:, :, 1:129], add,
                        )
                        nc.gpsimd.dma_start(
                            out_d.ap()[
                                img, m * 128 : (m + 1) * 128, r0 : r0 + nr, :
                            ],
                            o_t[:],
                        )

    nc.compile()
    return nc


def _compose_weights(w):
    """w (256,256,3,3) f32 -> wt (128, 4*12*128) f16.

    wh = w *h (1,1) (width 4), scaled 1/16. Layout [c_local, (m, bb, tap,
    o_local)]; tap order: e-plane k*4 + ds*2 + jj (ds=0 -> w row 0, 1 -> row
    2), o-plane 8 + k*2 + jj.  bb=0 uses wh col 2jj+1, bb=1 uses wh col 2jj.
    """
    w64 = w.astype(np.float64)
    wh = np.zeros((O, C, 3, 4), dtype=np.float64)
    wh[:, :, :, 0:3] += w64
    wh[:, :, :, 1:4] += w64
    wh *= 1.0 / 16.0

    wt = np.empty((128, 4 * NTAP * 128), dtype=np.float16)
    di_of_ds = (0, 2)
    for m in range(NM):
        for bb in range(2):
            base = (m * 2 + bb) * NTAP
            for k in range(NK):
                for ds in range(2):
                    for jj in range(2):
                        idx = base + k * 4 + ds * 2 + jj
                        sub = wh[m * 128 : (m + 1) * 128, k * 128 : (k + 1) * 128,
                                 di_of_ds[ds], 2 * jj + (1 - bb)]
                        wt[:, idx * 128 : (idx + 1) * 128] = sub.T.astype(np.float16)
                for jj in range(2):
                    idx = base + 8 + k * 2 + jj
                    sub = wh[m * 128 : (m + 1) * 128, k * 128 : (k + 1) * 128,
                             1, 2 * jj + (1 - bb)]
                    wt[:, idx * 128 : (idx + 1) * 128] = sub.T.astype(np.float16)
    return wt


def kernel(x, w, b):
    global _compiled, LAST_RESULTS
    if _compiled is None:
        _compiled = _build()
    nc = _compiled

    x = np.asarray(x, dtype=np.float32)
    w = np.asarray(w, dtype=np.float32)
    b = np.asarray(b, dtype=np.float32)

    wt = _compose_weights(w)
    b2 = np.ascontiguousarray((b / 32.0).reshape(NM, 128).T).astype(np.float32)
    xp = np.pad(x, ((0, 0), (0, 0), (1, 1), (1, 1))).astype(np.float16)
    xp = np.ascontiguousarray(xp.reshape(N_CORES, IMG_PER_CORE, NK, 128, HPR * HPC))

    in_maps = [{"xp": xp[core], "wt": wt, "bias": b2} for core in range(N_CORES)]
    try:
        res = run_bass_kernel_spmd(nc, in_maps, list(range(N_CORES)))
    except ModuleNotFoundError:
        import os

        os.environ["BASS_NEVER_TRACE"] = "1"
        res = run_bass_kernel_spmd(nc, in_maps, list(range(N_CORES)))
    LAST_RESULTS = res
    out = np.concatenate([res.results[i]["out"] for i in range(N_CORES)], axis=0)
    return out.astype(np.float32)


# revision 15
# speedup vs baseline: 1.1812x; 1.1812x over previous
"""StyleGAN2 fused upsample2x + 3x3 conv + FIR(1,3,3,1) + bias — TRN2 Bass kernel.

Decomposition: per dimension the reference is zero-insert by 2 -> correlate w
(full pad) -> pad 1 -> correlate FIR f1=(1,3,3,1)/4 (gain 2/dim).  f1 factors
as box^3/4 with box=(1,1).  We compose ONE box into w horizontally
(wh = w *h (1,1), width 4) and leave box^2=(1,2,1) horizontally plus box^3
vertically as cheap DVE adjacent-add cascades in fp16 (2x mode).

The conv produces the dense fine grid g[gt, gs] (132x130 alloc, 131x130 used):
  g rows: gt = 2t   (even)  = o-plane: w row 1   on x row u,  t = u+1
          gt = 2t+1 (odd)   = e-plane: w rows 0,2 on x rows u-1,u, t = u
  g cols: gs = 2v'+bb (g col gs = G[gs-1], G[t] = sum_j wh[j] xz[t+j-2]):
          bb=0 taps wh{1,3}, bb=1 taps wh{0,2}; both on x cols v'+jj-1
Then out[r,s] = sum (1,2,1)_h (1,3,3,1)_v g[r..r+3, s..s+2] (cascaded adds).
Weight scale 1/16 and bias/32 are folded in (h box^2 sums 4, v box^3 sums 8).

Matmuls fp16 (1 cycle/row).  K=256 in 2x128, o=256 in 2x128.  PSUM chains of
7 rows x 65 cols (455 fp32 <= 512/bank).  Data-parallel: 2 images/core.
"""

import sys

sys.path.insert(0, "/opt/trn_rl_repo")

import numpy as np

import concourse.bacc as bacc
import concourse.mybir as mybir
import concourse.tile as tile
from concourse.bass_utils import run_bass_kernel_spmd

N_CORES = 8
IMGS = 16
IMG_PER_CORE = IMGS // N_CORES  # 2
C = 256
O = 256
H = W = 64
NK = C // 128  # 2
NM = O // 128  # 2
HPR = 66  # padded x rows (pad 1 top/bottom)
HPC = 66  # padded x cols (pad 1 left/right)
GC = 130  # g cols used (alloc 130)
NTAP = 12  # taps per (m, bb): 8 e-plane + 4 o-plane
NU = 7  # chain rows
NV = 65  # chain cols (one col parity)

_compiled = None
LAST_RESULTS = None


def _chains(img, m):
    """Chain specs in execution order for one (img, m) plane.

    chain = (bb, two, t0, nu, taps); taps = (wt_idx, k, row_lo, col_lo):
      rhs = xp[img, k][:, row_lo : row_lo + nu, col_lo : col_lo + 65]
      g rows: 2*t + two for t in t0 .. t0+nu-1; g cols bb::2
    """
    for j in range(10):
        for plane in ("o", "e"):
            for bb in range(2):
                base = (m * 2 + bb) * NTAP
                if plane == "o":
                    u0 = -1 + NU * j
                    nu = min(NU, 65 - u0)  # u in -1..64 (66 rows)
                    if nu <= 0:
                        continue
                    taps = [
                        (base + 8 + k * 2 + jj, k, u0 + 1, jj)
                        for k in range(NK)
                        for jj in range(2)
                    ]
                    yield (bb, 0, u0 + 1, nu, taps)
                else:
                    u0 = NU * j
                    nu = min(NU, 65 - u0)  # u in 0..64 (65 rows)
                    if nu <= 0:
                        continue
                    taps = [
                        (base + k * 4 + ds * 2 + jj, k, u0 + ds, jj)
                        for k in range(NK)
                        for ds in range(2)  # ds=0 -> w row 0 (x[u-1]); 1 -> w row 2
                        for jj in range(2)
                    ]
                    yield (bb, 1, u0, nu, taps)


def _build():
    nc = bacc.Bacc(None, target_bir_lowering=False, debug=False)
    dt = mybir.dt

    xp_d = nc.dram_tensor(
        "xp", (IMG_PER_CORE, NK, 128, HPR * HPC), dt.float16, kind="ExternalInput"
    )
    wt_d = nc.dram_tensor(
        "wt", (128, 4 * NTAP * 128), dt.float16, kind="ExternalInput"
    )
    b_d = nc.dram_tensor("bias", (128, NM), dt.float32, kind="ExternalInput")
    out_d = nc.dram_tensor(
        "out", (IMG_PER_CORE, O, 2 * H, 2 * W), dt.float16, kind="ExternalOutput"
    )

    with tile.TileContext(nc) as tc:
        with (
            tc.tile_pool(name="xpool", bufs=1) as xpool,
            tc.tile_pool(name="wpool", bufs=1) as wpool,
            tc.tile_pool(name="gpool", bufs=2) as gpool,
            tc.tile_pool(name="cpool", bufs=1) as cpool,
            tc.tile_pool(name="opool", bufs=3) as opool,
            tc.tile_pool(name="wpsum", bufs=1, space="PSUM") as wpsum_pool,
            tc.tile_pool(name="psum", bufs=7, space="PSUM") as psum_pool,
        ):
            wt_t = wpool.tile([128, 4 * NTAP * 128], dt.float16, tag="wt")
            b_t = wpool.tile([128, NM], dt.float32, tag="bias")
            xp_t = {}

            def load_xp(img, k, strips):
                if (img, k) not in xp_t:
                    xp_t[img, k] = xpool.tile(
                        [128, HPR, HPC], dt.float16, tag=f"xp{img}{k}",
                        name=f"xp{img}{k}",
                    )
                t = xp_t[img, k]
                src = xp_d.ap()[img, k].rearrange("p (h w) -> p h w", h=HPR)
                for lo, hi in strips:
                    nc.sync.dma_start(t[:, lo:hi, :], src[:, lo:hi, :])

            def load_wt(m):
                blk = m * 2 * NTAP * 128
                n = 2 * NTAP * 128
                nc.sync.dma_start(wt_t[:, blk : blk + n], wt_d.ap()[:, blk : blk + n])

            # PE warmup during DMA preamble: garbage matmuls ramp the
            # tensor engine p-state to full clock before real work arrives.
            warm_t = wpool.tile([128, 512], dt.float16, name="warm")
            nc.gpsimd.memset(warm_t[:], 0.0)
            warm_ps = wpsum_pool.tile([128, 512], dt.float32, name="warm_ps")
            for _ in range(14):
                nc.tensor.matmul(
                    warm_ps[:], warm_t[:, :128], warm_t[:], start=True, stop=True
                )
            # minimal working set first: j=0..1 chains need xp rows < 16
            load_wt(0)
            load_xp(0, 0, [(0, 16)])
            load_xp(0, 1, [(0, 16)])
            nc.sync.dma_start(b_t[:], b_d.ap()[:])
            load_xp(0, 0, [(16, HPR)])
            load_xp(0, 1, [(16, HPR)])
            load_wt(1)
            load_xp(1, 0, [(0, HPR)])
            load_xp(1, 1, [(0, HPR)])

            for img in range(IMG_PER_CORE):
                for m in range(NM):
                    g_t = gpool.tile([128, 132, GC], dt.float16, name="g")
                    g_v = g_t[:].rearrange(
                        "p (t two) (v bb) -> p two t v bb", two=2, bb=2
                    )
                    for bb, two, t0, nu, taps in _chains(img, m):
                        acc = psum_pool.tile([128, nu, NV], dt.float32, name="acc")
                        for i, (wt_idx, k, row_lo, col_lo) in enumerate(taps):
                            nc.tensor.matmul(
                                acc[:],
                                wt_t[:, wt_idx * 128 : (wt_idx + 1) * 128],
                                xp_t[img, k][:, row_lo : row_lo + nu, col_lo : col_lo + NV],
                                start=(i == 0),
                                stop=(i == len(taps) - 1),
                            )
                        nc.scalar.activation(
                            g_v[:, two, t0 : t0 + nu, :, bb],
                            acc[:],
                            mybir.ActivationFunctionType.Identity,
                            bias=b_t[:, m : m + 1],
                        )
                    # FIR cascades: v (1,3,3,1) then h (1,2,1), fp16 adds.
                    # bands = (r0, nrows); v2 of bands 0,2 runs on GpSimd.
                    first_unit = img == 0 and m == 0
                    last_unit = img == IMG_PER_CORE - 1 and m == NM - 1
                    bands = (
                        [(0, 16), (16, 16)] if first_unit else [(0, 32)]
                    ) + [(32, 32), (64, 32)] + (
                        [(96, 16), (112, 16)] if last_unit else [(96, 32)]
                    )
                    add = mybir.AluOpType.add
                    for bi, (r0, nr) in enumerate(bands):
                        p1 = cpool.tile([128, nr + 2, GC], dt.float16, name="casc_p1")
                        p2 = cpool.tile([128, nr + 1, GC], dt.float16, name="casc_p2")
                        p3 = cpool.tile([128, nr, GC], dt.float16, name="casc_p3")
                        p4 = cpool.tile([128, nr, 129], dt.float16, name="casc_p4")
                        o_t = opool.tile([128, nr, 128], dt.float16, name="casc_o")
                        nc.vector.tensor_tensor(
                            p1[:], g_t[:, r0 : r0 + nr + 2, :GC],
                            g_t[:, r0 + 1 : r0 + nr + 3, :GC], add,
                        )
                        nc.vector.tensor_tensor(
                            p2[:], p1[:, 0 : nr + 1, :], p1[:, 1 : nr + 2, :], add,
                        )
                        nc.vector.tensor_tensor(
                            p3[:], p2[:, 0:nr, :], p2[:, 1 : nr + 1, :], add,
                        )
                        nc.vector.tensor_tensor(
                            p4[:], p3[:, :, 0:129], p3[:, :, 1:130], add,
                        )
                        nc.vector.tensor_tensor(
                            o_t[:], p4[:, :, 0:128], p4[:, :, 1:129], add,
                        )
                        nc.gpsimd.dma_start(
                            out_d.ap()[
                                img, m * 128 : (m + 1) * 128, r0 : r0 + nr, :
                            ],
                            o_t[:],
                        )

    nc.compile()
    return nc


def _compose_weights(w):
    """w (256,256,3,3) f32 -> wt (128, 4*12*128) f16.

    wh = w *h (1,1) (width 4), scaled 1/16. Layout [c_local, (m, bb, tap,
    o_local)]; tap order: e-plane k*4 + ds*2 + jj (ds=0 -> w row 0, 1 -> row
    2), o-plane 8 + k*2 + jj.  bb=0 uses wh col 2jj+1, bb=1 uses wh col 2jj.
    """
    w64 = w.astype(np.float64)
    wh = np.zeros((O, C, 3, 4), dtype=np.float64)
    wh[:, :, :, 0:3] += w64
    wh[:, :, :, 1:4] += w64
    wh *= 1.0 / 16.0

    wt = np.empty((128, 4 * NTAP * 128), dtype=np.float16)
    di_of_ds = (0, 2)
    for m in range(NM):
        for bb in range(2):
            base = (m * 2 + bb) * NTAP
            for k in range(NK):
                for ds in range(2):
                    for jj in range(2):
                        idx = base + k * 4 + ds * 2 + jj
                        sub = wh[m * 128 : (m + 1) * 128, k * 128 : (k + 1) * 128,
                                 di_of_ds[ds], 2 * jj + (1 - bb)]
                        wt[:, idx * 128 : (idx + 1) * 128] = sub.T.astype(np.float16)
                for jj in range(2):
                    idx = base + 8 + k * 2 + jj
                    sub = wh[m * 128 : (m + 1) * 128, k * 128 : (k + 1) * 128,
                             1, 2 * jj + (1 - bb)]
                    wt[:, idx * 128 : (idx + 1) * 128] = sub.T.astype(np.float16)
    return wt


def kernel(x, w, b):
    global _compiled, LAST_RESULTS
    if _compiled is None:
        _compiled = _build()
    nc = _compiled

    x = np.asarray(x, dtype=np.float32)
    w = np.asarray(w, dtype=np.float32)
    b = np.asarray(b, dtype=np.float32)

    wt = _compose_weights(w)
    b2 = np.ascontiguousarray((b / 32.0).reshape(NM, 128).T).astype(np.float32)
    xp = np.pad(x, ((0, 0), (0, 0), (1, 1), (1, 1))).astype(np.float16)
    xp = np.ascontiguousarray(xp.reshape(N_CORES, IMG_PER_CORE, NK, 128, HPR * HPC))

    in_maps = [{"xp": xp[core], "wt": wt, "bias": b2} for core in range(N_CORES)]
    try:
        res = run_bass_kernel_spmd(nc, in_maps, list(range(N_CORES)))
    except ModuleNotFoundError:
        import os

        os.environ["BASS_NEVER_TRACE"] = "1"
        res = run_bass_kernel_spmd(nc, in_maps, list(range(N_CORES)))
    LAST_RESULTS = res
    out = np.concatenate([res.results[i]["out"] for i in range(N_CORES)], axis=0)
    return out.astype(np.float32)
